# revision 20
# baseline (speedup 1.0000x reference)
"""Trainium2 Bass kernel for CRF loss (MLP emissions + CRF log-likelihood).

Sharding: data-parallel over B=256 sentences -> 32 per core on 8 cores.
Sentences are globally sorted by length (desc) and dealt round-robin to
cores so every core shares one "active-eighth profile" (ceil(len/64)
eighths per slot) -> a single SPMD module skips padding work uniformly.

Per core:
  MLP: fp8 (e4m3) DoubleRow matmuls. Only active eighths computed.
  em transport: per sentence-pair, PSUM em is evacuated to a tiny SBUF
  staging tile, then ONE SBUF->SBUF DMA redistributes it into the CRF
  lane layout (no DRAM round-trip).
  CRF: per-(sentence, eighth) lane layout (128 partitions x 2 halves),
  transfer-matrix binary tree over 64 steps in the free dim, then a
  stream_shuffle tree folds the 8 eighths/sentence. exp(trans)/3 keeps
  the rescale-free tree in fp32 range (compensated by -ln3 per active
  transition in the numerator constants). All tag/length-only terms are
  reduced in a prep pass that hides under the MLP.
  Schedule: the SHORT half of the batch runs its MLP first, so its CRF
  chain hides under the long half's MLP; the long half's chain is the
  only exposed tail.
"""

import sys

sys.path.insert(0, "/opt/trn_rl_repo")

import numpy as np
import ml_dtypes
from contextlib import ExitStack

import concourse.bass as bass
import concourse.mybir as mybir
import concourse.tile as tile
from concourse import bass_utils

F32 = mybir.dt.float32
FP8 = mybir.dt.float8e4
I32 = mybir.dt.int32
AF = mybir.ActivationFunctionType
OP = mybir.AluOpType
AX = mybir.AxisListType
DR = mybir.MatmulPerfMode.DoubleRow

BS, T, D, H, K = 32, 512, 512, 256, 3  # per-core shard
NCORES = 8
NE8 = 8          # eighths per sentence
TE = 64          # tokens per eighth
SC = 64.0        # fp8 weight scale
LN3 = float(np.log(3.0))

PUMP_RATE = 5    # generator steps per MLP slot while pumping
DEBUG_EM = False


def build(trans, start, end, b1, b2, na_prof):
    trans = np.asarray(trans, np.float64)
    start = np.asarray(start, np.float64)
    end = np.asarray(end, np.float64)
    b1 = np.asarray(b1, np.float64)
    b2 = np.asarray(b2, np.float64)
    assert np.all(b1 == 0.0), "b1 != 0 unsupported fast path"
    assert np.all(b2 == 0.0), "b2 != 0 unsupported fast path"
    na_prof = [int(v) for v in na_prof]
    NE = int(sum(na_prof))
    q0 = np.concatenate([[0], np.cumsum(na_prof)]).astype(int)

    nc = bass.Bass()
    xall_d = nc.dram_tensor("xall", [128, 4, NE, TE], FP8, kind="ExternalInput")
    w1_d = nc.dram_tensor("w1q", [128, 4, H], FP8, kind="ExternalInput")
    w2_d = nc.dram_tensor("w2q", [128, 2, 32], FP8, kind="ExternalInput")
    tg_d = nc.dram_tensor("tags", [BS, T], I32, kind="ExternalInput")
    ln_d = nc.dram_tensor("lengths", [BS], I32, kind="ExternalInput")
    out_d = nc.dram_tensor("out", [2, 128], F32, kind="ExternalOutput")
    lnx_dram = nc.dram_tensor("lnx_scratch", [2, 128, 2], F32, kind="Internal")
    em_dram = nc.dram_tensor("em_scratch", [BS * NE8, K, TE], F32, kind="Internal")
    dbg_d = nc.dram_tensor("dbg", [2, 128, K, TE], F32, kind="ExternalOutput") \
        if DEBUG_EM else None

    ex_trans3 = np.exp(trans + b2[None, :]) / 3.0
    ex_end = np.exp(end)
    tfp = trans.reshape(9) - LN3

    with tile.TileContext(nc) as tc, ExitStack() as ctx:
        consts = ctx.enter_context(tc.tile_pool(name="consts", bufs=1))
        ps_h = ctx.enter_context(tc.tile_pool(name="ps_h", bufs=2, space="PSUM"))
        ps_e = ctx.enter_context(tc.tile_pool(name="ps_e", bufs=2, space="PSUM"))
        esb_p = ctx.enter_context(tc.tile_pool(name="esb", bufs=2))
        tree_p = ctx.enter_context(tc.tile_pool(name="tree", bufs=2))
        sm_p = ctx.enter_context(tc.tile_pool(name="small", bufs=2))

        # ---------------- weights + x chunks (SP HWDGE queue) --------------
        w1q = consts.tile([128, 4, H], FP8)
        nc.sync.dma_start(w1q[:], w1_d[:])
        w2q = consts.tile([128, 2, 32], FP8)
        nc.sync.dma_start(w2q[:], w2_d[:])
        xall = consts.tile([128, 4, NE, TE], FP8)

        chunk_order = [4, 5, 6, 7, 0, 1, 2, 3]  # short half first

        def load_chunk(c):
            blo, bhi = 4 * c, 4 * (c + 1)
            slo, shi = int(q0[blo]), int(q0[bhi])
            if shi > slo:
                nc.sync.dma_start(xall[:, :, slo:shi, :],
                                   xall_d[:, :, slo:shi, :])

        load_chunk(chunk_order[0])
        load_chunk(chunk_order[1])
        # early tiny DMAs: tags + broadcast lengths (via DRAM round-trip)
        tg_t = [None, None]
        ln_t = [None, None]
        for h in (1, 0):
            tg_i = consts.tile([128, TE], I32, name=f"tg128_{h}")
            nc.sync.dma_start(
                tg_i[:],
                tg_d[16 * h:16 * h + 16].rearrange("b (e t) -> (b e) t", e=NE8))
            tg_t[h] = tg_i
            li_h = consts.tile([16, 1], I32, name=f"li{h}")
            nc.sync.dma_start(
                li_h[:], ln_d[16 * h:16 * h + 16].rearrange("(b o) -> b o", o=1))
            lif = consts.tile([16, 1], F32, name=f"lif{h}")
            nc.vector.tensor_copy(lif[:], li_h[:])
            lib = consts.tile([16, NE8, 2], F32, name=f"lib{h}")
            nc.vector.tensor_copy(lib[:, :, 0],
                                  lif[:].broadcast_to((16, NE8)))
            ei_h = consts.tile([16, NE8], I32, name=f"ei{h}")
            nc.gpsimd.iota(ei_h[:], pattern=[[1, NE8]], base=0,
                           channel_multiplier=0)
            nc.vector.tensor_copy(lib[:, :, 1], ei_h[:])
            nc.sync.dma_start(
                lnx_dram[h].rearrange("(b e) c -> b (e c)", e=NE8), lib[:])
        load_chunk(chunk_order[2])
        for h in (1, 0):
            lni = consts.tile([128, 2], F32, name=f"lni{h}")
            nc.sync.dma_start(lni[:], lnx_dram[h])
            ln_t[h] = lni

        # ---------------- constants ----------------
        Kc = consts.tile([128, 9], F32)
        for i in range(K):
            for j in range(K):
                nc.gpsimd.memset(Kc[:, 3 * i + j:3 * i + j + 1],
                                 float(ex_trans3[i, j]))
        startc = consts.tile([128, 3], F32)
        eendc = consts.tile([128, 3], F32)
        for j in range(K):
            nc.gpsimd.memset(startc[:, j:j + 1], float(start[j] + b2[j]))
            nc.gpsimd.memset(eendc[:, j:j + 1], float(ex_end[j]))
        it_i = consts.tile([128, TE], I32)
        nc.gpsimd.iota(it_i[:], pattern=[[1, TE]], base=0, channel_multiplier=0)
        itf = consts.tile([128, TE], F32)
        nc.gpsimd.tensor_copy(itf[:], it_i[:])

        # em staging: per-pair SBUF tile -> DRAM (lane-major) -> SBUF lanes
        em128 = [consts.tile([128, K, TE], F32, name=f"em128_{h}")
                 for h in (0, 1)]
        esb_bufs = []
        for r in range(2):
            e = esb_p.tile([K, 2, NE8, TE], F32, tag="esb")
            nc.vector.memset(e[:], 0.0)
            esb_bufs.append(e)

        half = [dict(), dict()]
        for h in (1, 0):
            tgf = consts.tile([128, TE], F32, name=f"tgf_{h}")
            nc.vector.tensor_copy(tgf[:], tg_t[h][:])
            tg0sh = consts.tile([128, 1], F32, name=f"tg0sh_{h}")
            nc.vector.stream_shuffle(tg0sh[:], tgf[:, 0:1],
                                     [(i + 1) % 32 for i in range(32)])
            half[h]["tgf"] = tgf
            half[h]["tg0sh"] = tg0sh

        # ------------- per-half tag/length prep (no em needed) -------------
        def crf_pre(h, eng):
            st = half[h]
            emf = ln_t[h][:, 1:2]
            tgf = st["tgf"]
            lnc = sm_p.tile([128, 1], F32, tag=f"lnc{h}")
            nc.vector.tensor_scalar_max(lnc[:], ln_t[h][:, 0:1], 1.0)
            lq = consts.tile([128, 2], F32, name=f"lq128_{h}")
            nc.vector.scalar_tensor_tensor(lq[:, 0:1], emf, -64.0, lnc[:],
                                     OP.mult, OP.add)
            nc.vector.tensor_scalar(lq[:, 1:2], emf, 0.0, None, OP.is_equal)
            lqc = lq[:, 0:1]
            e0 = lq[:, 1:2]
            m1b = consts.tile([128, TE], F32, name=f"m1b_{h}")
            nc.vector.tensor_scalar(m1b[:], itf[:], lqc, None, OP.is_lt)
            mge = sm_p.tile([128, TE], F32, tag=f"mge{h}")
            nc.vector.tensor_scalar(mge[:], itf[:], e0, None, OP.is_ge)
            mpb = consts.tile([128, TE], F32, name=f"mpb_{h}")
            eng.tensor_mul(mpb[:], m1b[:], mge[:])
            ohm = consts.tile([128, K, TE], F32, name=f"ohm_{h}")
            for j in range(K):
                nc.vector.scalar_tensor_tensor(
                    ohm[:, j, :], tgf[:], float(j), m1b[:],
                    OP.is_equal, OP.mult)
            idx = sm_p.tile([128, TE], F32, tag=f"idx{h}")
            nc.vector.scalar_tensor_tensor(
                idx[:, 1:TE], tgf[:, 0:TE - 1], 3.0, tgf[:, 1:TE],
                OP.mult, OP.add)
            nc.vector.scalar_tensor_tensor(
                idx[:, 0:1], tgf[:, TE - 1:TE], 3.0, st["tg0sh"][:],
                OP.mult, OP.add)
            tr = sm_p.tile([128, TE], F32, tag=f"tr{h}")
            nc.vector.tensor_scalar(tr[:], idx[:], 0.0, float(tfp[0]),
                              OP.is_equal, OP.mult)
            for p in range(1, 9):
                u = sm_p.tile([128, TE], F32, tag=f"trsel{h}")
                nc.vector.tensor_scalar(u[:], idx[:], float(p), float(tfp[p]),
                                  OP.is_equal, OP.mult)
                eng.tensor_add(tr[:], tr[:], u[:])
            trm = sm_p.tile([128, TE], F32, tag=f"trm{h}")
            eng.tensor_copy(trm[:, 1:TE], mpb[:, 1:TE])
            nc.vector.tensor_scalar(trm[:, 0:1], lqc, 64.0, None, OP.is_gt)
            # trq = sum(tr * trm)  (numerator transition score, -ln3-adjusted)
            trs = sm_p.tile([128, TE], F32, tag=f"trs{h}")
            trq = sm_p.tile([128, 1], F32, tag=f"trq{h}")
            eng.tensor_mul(trs[:], tr[:], trm[:])
            nc.vector.tensor_reduce(trq[:], trs[:], axis=AX.X, op=OP.add)
            # last-tag end-term, fully per-lane (only the lane holding the
            # last token contributes; summed across lanes by the folds)
            indL = sm_p.tile([128, TE], F32, tag=f"indL{h}")
            nc.vector.tensor_scalar(indL[:], itf[:], lqc, -1.0,
                              OP.subtract, OP.is_equal)
            ltsx = sm_p.tile([128, TE], F32, tag=f"ltsx{h}")
            ltsr = sm_p.tile([128, 1], F32, tag=f"ltsr{h}")
            eng.tensor_mul(ltsx[:], tgf[:], indL[:])
            nc.vector.tensor_reduce(ltsr[:], ltsx[:], axis=AX.X, op=OP.add)
            indr = sm_p.tile([128, 1], F32, tag=f"indr{h}")
            nc.vector.tensor_reduce(indr[:], indL[:], axis=AX.X, op=OP.add)
            c1v = float((4.0 * end[1] - 3.0 * end[0] - end[2]) / 2.0)
            c2v = float((end[2] - 2.0 * end[1] + end[0]) / 2.0)
            lt2 = sm_p.tile([128, 1], F32, tag=f"lt2{h}")
            eng.tensor_mul(lt2[:], ltsr[:], ltsr[:])
            eu = sm_p.tile([128, 1], F32, tag=f"eu{h}")
            nc.vector.tensor_scalar(eu[:], ltsr[:], c1v, None, OP.mult)
            ev = sm_p.tile([128, 1], F32, tag=f"ev{h}")
            nc.vector.scalar_tensor_tensor(ev[:], lt2[:], c2v, eu[:],
                                     OP.mult, OP.add)
            ew = sm_p.tile([128, 1], F32, tag=f"ew{h}")
            nc.vector.scalar_tensor_tensor(ew[:], indr[:], float(end[0]), ev[:],
                                     OP.mult, OP.add)
            # first-tag start-term (e0 lanes only)
            fa = sm_p.tile([128, 1], F32, tag=f"fa{h}")
            nc.vector.tensor_scalar(fa[:], tgf[:, 0:1], 0.0,
                              float(start[0]), OP.is_equal, OP.mult)
            for j in (1, 2):
                fb = sm_p.tile([128, 1], F32, tag=f"fb{h}")
                nc.vector.tensor_scalar(fb[:], tgf[:, 0:1], float(j),
                                  float(start[j]), OP.is_equal, OP.mult)
                eng.tensor_add(fa[:], fa[:], fb[:])
            fae = sm_p.tile([128, 1], F32, tag=f"fae{h}")
            eng.tensor_mul(fae[:], fa[:], e0)
            # trqp = trq + end-term + start-term  (numerator, em-free part)
            trqp = consts.tile([128, 1], F32, name=f"trqp_{h}")
            eng.tensor_add(trqp[:], trq[:], ew[:])
            eng.tensor_add(trqp[:], trqp[:], fae[:])
            e0q = consts.tile([128, 1], F32, name=f"e0q_{h}")
            nc.vector.tensor_scalar_mul(e0q[:], e0, 1.0 / SC)
            # Km2 = mpb*Kc + (1-mpb)*I  (masked transition matrices)
            Km = consts.tile([128, TE, 9], F32, name=f"Km_{h}")
            eng.tensor_mul(Km[:], mpb[:].unsqueeze(2).broadcast_to(
                (128, TE, 9)), Kc[:].unsqueeze(1).broadcast_to((128, TE, 9)))
            omm = sm_p.tile([128, TE], F32, tag=f"omm{h}")
            nc.vector.tensor_scalar(omm[:], mpb[:], -1.0, 1.0, OP.mult, OP.add)
            for j in range(K):
                eng.tensor_add(Km[:, :, 4 * j], Km[:, :, 4 * j], omm[:])
            st.update(Km=Km, ohm=ohm, trqp=trqp, e0q=e0q, mpb=mpb)

        # ------------- per-half em-dependent CRF chain (generator) ---------
        def crf_main(h):
            st = half[h]
            em = em128[h]
            nc.sync.dma_start(em[:], em_dram[128 * h:128 * h + 128])
            yield
            # zero masked em so exp -> 1 there (Km2 identity then holds)
            emm = sm_p.tile([128, K, TE], F32, tag=f"emm{h}")
            nc.vector.tensor_mul(
                emm[:], em[:],
                st["mpb"][:].unsqueeze(1).broadcast_to((128, K, TE)))
            yield
            E = sm_p.tile([128, K, TE], F32, tag=f"E{h}")
            nc.scalar.activation(E[:], emm[:], AF.Exp, scale=1.0 / SC)
            yield
            M0 = tree_p.tile([128, TE, 9], F32, tag=f"M0_{h}")
            nc.vector.tensor_mul(
                M0[:].rearrange("p t (i j) -> p t i j", i=3),
                E[:].rearrange("p j t -> p t j").unsqueeze(2)
                    .broadcast_to((128, TE, 3, 3)),
                st["Km"][:].rearrange("p t (i j) -> p t i j", i=3))
            yield
            cur = M0
            curN = TE
            while curN > 1:
                N = curN // 2
                A_v = cur[:, 0:curN, :].rearrange(
                    "p (n two) e -> p n two e", two=2)[:, :, 0, :].rearrange(
                    "p n (a k) -> p n a k", a=3)
                B_v = cur[:, 0:curN, :].rearrange(
                    "p (n two) e -> p n two e", two=2)[:, :, 1, :].rearrange(
                    "p n (k b) -> p n k b", k=3)
                tmps = []
                for kk in range(3):
                    tm = tree_p.tile([128, N, 9], F32, tag=f"tmp{h}_{N}_{kk}")
                    tv = tm[:].rearrange("p n (a b) -> p n a b", a=3)
                    Ak = A_v[:, :, :, kk].unsqueeze(3)
                    Bk = B_v[:, :, kk, :].unsqueeze(2)
                    nc.vector.tensor_mul(
                        tv[:], Ak[:].broadcast_to((128, N, 3, 3)),
                        Bk[:].broadcast_to((128, N, 3, 3)))
                    tmps.append(tm)
                    yield
                nxt = tree_p.tile([128, N, 9], F32, tag=f"nxt{h}_{N}")
                nc.vector.tensor_add(nxt[:], tmps[0][:], tmps[1][:])
                yield
                nc.vector.tensor_add(nxt[:], nxt[:], tmps[2][:])
                yield
                cur, curN = nxt, N
            # rescale the per-eighth product; log rides in pay[9]
            pay = consts.tile([128, 16], F32, name=f"pay_{h}")
            mx = sm_p.tile([128, 1], F32, tag=f"mx{h}")
            nc.vector.reduce_max(mx[:], cur[:, 0, :], axis=AX.X)
            yield
            rc = sm_p.tile([128, 1], F32, tag=f"rc{h}")
            nc.vector.reciprocal(rc[:], mx[:])
            yield
            nc.vector.tensor_scalar(pay[:, 0:9], cur[:, 0, :], rc[:, 0:1],
                                    None, OP.mult)
            yield
            nc.scalar.activation(pay[:, 9:10], mx[:], AF.Ln)
            yield
            # numerator: gold emissions + prep terms
            ems = sm_p.tile([128, K * TE], F32, tag=f"ems{h}")
            nc.vector.tensor_mul(ems[:], em[:].rearrange("p k t -> p (k t)"),
                                 st["ohm"][:].rearrange("p k t -> p (k t)"))
            yield
            emt = sm_p.tile([128, 1], F32, tag=f"emt{h}")
            nc.vector.tensor_reduce(emt[:], ems[:], axis=AX.X, op=OP.add)
            yield
            nc.vector.scalar_tensor_tensor(pay[:, 10:11], emt[:], 1.0 / SC,
                                           st["trqp"][:], OP.mult, OP.add)
            yield
            nc.vector.tensor_scalar(pay[:, 11:14], em[:, :, 0], st["e0q"][:, 0:1],
                                    None, OP.mult)
            yield
            curp = pay
            for k in (1, 2, 4):
                shp = sm_p.tile([128, 16], F32, tag=f"shp{h}{k}")
                nc.vector.stream_shuffle(shp[:, 0:14], curp[:, 0:14],
                                         [(i + k) % 32 for i in range(32)])
                yield
                nxtp = sm_p.tile([128, 16], F32, tag=f"nxtp{h}{k}")
                tmf = sm_p.tile([128, 3, 3, 3], F32, tag=f"tmpf{h}{k}")
                nc.vector.tensor_mul(
                    tmf[:],
                    curp[:, 0:9].rearrange("p (a k2) -> p a k2", a=3)
                        .unsqueeze(2).broadcast_to((128, 3, 3, 3)),
                    shp[:, 0:9].rearrange("p (k2 b) -> p k2 b", k2=3)
                        .unsqueeze(1).broadcast_to((128, 3, 3, 3)))
                yield
                nc.vector.tensor_add(nxtp[:, 0:9],
                                     tmf[:, :, :, 0].rearrange(
                                         "p a b -> p (a b)"),
                                     tmf[:, :, :, 1].rearrange(
                                         "p a b -> p (a b)"))
                yield
                nc.vector.tensor_add(nxtp[:, 0:9], nxtp[:, 0:9],
                                     tmf[:, :, :, 2].rearrange(
                                         "p a b -> p (a b)"))
                yield
                nc.vector.tensor_add(nxtp[:, 9:14], curp[:, 9:14],
                                     shp[:, 9:14])
                yield
                curp = nxtp
            s0 = sm_p.tile([128, 3], F32, tag=f"s0{h}")
            nc.vector.tensor_add(s0[:], curp[:, 11:14], startc[:])
            yield
            a0 = sm_p.tile([128, 3], F32, tag=f"a0{h}")
            nc.scalar.activation(a0[:], s0[:], AF.Exp)
            yield
            w9 = sm_p.tile([128, 3, 3], F32, tag=f"w9{h}")
            nc.vector.tensor_mul(
                w9[:], a0[:].unsqueeze(2).broadcast_to((128, 3, 3)),
                eendc[:].unsqueeze(1).broadcast_to((128, 3, 3)))
            yield
            zs = sm_p.tile([128, 9], F32, tag=f"zs{h}")
            nc.vector.tensor_mul(zs[:], curp[:, 0:9],
                                 w9[:].rearrange("p a b -> p (a b)"))
            yield
            zv = sm_p.tile([128, 1], F32, tag=f"zv{h}")
            nc.vector.tensor_reduce(zv[:], zs[:], axis=AX.X, op=OP.add)
            yield
            lgz = sm_p.tile([128, 1], F32, tag=f"lgz{h}")
            nc.scalar.activation(lgz[:], zv[:], AF.Ln)
            yield
            den = sm_p.tile([128, 1], F32, tag=f"den{h}")
            nc.vector.tensor_add(den[:], lgz[:], curp[:, 9:10])
            yield
            llh = sm_p.tile([128, 1], F32, tag=f"llh{h}")
            nc.vector.tensor_sub(llh[:], curp[:, 10:11], den[:])
            yield
            nc.sync.dma_start(out_d[h].rearrange("(p o) -> p o", o=1), llh[:])
            yield

        # prep for the first-processed (short, h=1) half up front
        crf_pre(1, nc.vector)

        # ---------------- MLP loop -----------------------------------------
        gens = []
        crf_band = [50]

        def pump(n):
            old = tc.cur_priority
            tc.cur_priority = crf_band[0]
            for g in list(gens):
                for _ in range(n):
                    try:
                        next(g)
                    except StopIteration:
                        gens.remove(g)
                        break
            crf_band[0] = tc.cur_priority
            tc.cur_priority = old

        gt = [consts.tile([128, 2, T], FP8, name=f"gbuf{r}") for r in range(3)]
        proc_order = list(range(16, 32)) + list(range(16))
        for bi, b in enumerate(proc_order):
            na = na_prof[b]
            nt = na * TE
            s4 = b % 4
            if s4 == 0 and bi // 4 + 3 < 8:
                load_chunk(chunk_order[bi // 4 + 3])
            if b % 2 == 0:
                pe = ps_e.tile([32, 2 * T], F32, tag="pe")
            sl = slice(int(q0[b]), int(q0[b + 1]))
            ph = ps_h.tile([128, 2, T], F32, tag="ph")
            for ht in range(2):
                for dcp in range(2):
                    nc.tensor.matmul(
                        ph[:, ht, 0:nt],
                        lhsT=w1q[:, 2 * dcp:2 * dcp + 2, 128 * ht:128 * (ht + 1)],
                        rhs=xall[:, 2 * dcp:2 * dcp + 2, sl, :].rearrange(
                            "p c q t -> p c (q t)"),
                        start=(dcp == 0), stop=(dcp == 1), perf_mode=DR)
            g = gt[b % 3]
            nc.scalar.activation(g[:, :, 0:nt], ph[:, :, 0:nt], AF.Gelu,
                                 scale=1.0 / SC)
            p2 = b % 2
            nc.tensor.matmul(pe[:, p2 * T:p2 * T + nt],
                             lhsT=w2q[:], rhs=g[:, :, 0:nt],
                             start=True, stop=True, perf_mode=DR)
            if p2 == 1:
                # evacuate the pair's em from PSUM and store it to DRAM in
                # lane-major order (load per half brings it back to lanes)
                nap = na_prof[b - 1]
                ntp = nap * TE
                esb = esb_bufs[(bi // 2) % 2]
                if ntp == nt:
                    nc.vector.tensor_copy(
                        esb[:, :, 0:na, :],
                        pe[0:K].rearrange("k (p t) -> k p t", p=2)[:, :, 0:nt]
                            .rearrange("k p (e t) -> k p e t", t=TE))
                else:
                    nc.vector.tensor_copy(
                        esb[:, 0, 0:nap, :],
                        pe[0:K, 0:ntp].rearrange("k (e t) -> k e t", t=TE))
                    nc.vector.tensor_copy(
                        esb[:, 1, 0:na, :],
                        pe[0:K, T:T + nt].rearrange("k (e t) -> k e t", t=TE))
                r0 = (b // 16) * 128 + ((b - 1) % 16) * 8
                nc.sync.dma_start(
                    em_dram[r0:r0 + 16].rearrange("l k t -> k l t"),
                    esb[:].rearrange("k p e t -> k (p e) t"))
            if bi == 15:
                gens.append(crf_main(1))
            if bi == 18:
                old = tc.cur_priority
                tc.cur_priority = crf_band[0]
                crf_pre(0, nc.gpsimd)
                crf_band[0] = tc.cur_priority
                tc.cur_priority = old
            if bi >= 16:
                pump(PUMP_RATE)
        gens.append(crf_main(0))
        pump(1000)
        if DEBUG_EM:
            for h in (0, 1):
                dtile = consts.tile([128, K, TE], F32, name=f"dbg_{h}")
                nc.vector.tensor_copy(dtile[:], em128[h][:])
                nc.sync.dma_start(dbg_d[h], dtile[:])

    return nc


def split_waits(nc, max_waits=1):
    """Walrus accepts only one sync-wait per instruction; move extra waits
    onto same-engine NoOps (engines execute in order)."""
    n = 0
    for f in nc.m.functions:
        for blk in f.blocks:
            new_insts = []
            for inst in blk.instructions:
                si = getattr(inst, "sync_info", None)
                waits = list(si.on_wait) if si is not None and si.on_wait else []
                if len(waits) > max_waits:
                    for w in waits[:-max_waits]:
                        n += 1
                        nop = mybir.InstNoOp(name=f"W-{n}", ins=[], outs=[])
                        nop.engine = inst.engine
                        nop.sync_info = mybir.SyncInfo(on_wait=[w], on_update=[])
                        new_insts.append(nop)
                    si.on_wait = waits[-max_waits:]
                new_insts.append(inst)
            try:
                blk.instructions = new_insts
            except Exception:
                blk.instructions[:] = new_insts
    return n


def plan(lengths):
    lengths = np.maximum(np.asarray(lengths, np.int64), 1)
    na = np.minimum((lengths + TE - 1) // TE, NE8)
    order = np.argsort(-na, kind="stable")
    rows = order.reshape(BS, NCORES)          # rank-row j -> 8 global ids
    assign = rows[np.arange(BS)]
    na_prof = na[assign[:, 0]]
    return assign, na_prof


def pack_inputs(x, tags, lengths, na_prof, assign):
    B = x.shape[0]
    na_prof = np.asarray(na_prof, np.int64)
    NE = int(na_prof.sum())
    in_maps = []
    xr = x.reshape(B, NE8, TE, D)
    for c in range(NCORES):
        gids = assign[:, c]
        xs = np.empty((NE, TE, D), np.float32)
        o = 0
        for j, g in enumerate(gids):
            n = int(na_prof[j])
            xs[o:o + n] = xr[g, :n]
            o += n
        xq = np.ascontiguousarray(
            xs.transpose(2, 0, 1).reshape(4, 128, NE, TE).transpose(1, 0, 2, 3)
        ).astype(ml_dtypes.float8_e4m3)
        in_maps.append({
            "xall": xq,
            "tags": np.ascontiguousarray(tags[gids], np.int32),
            "lengths": np.ascontiguousarray(lengths[gids], np.int32),
        })
    return in_maps


def quant_weights(W1, W2):
    w1q = np.ascontiguousarray(
        (np.asarray(W1, np.float64) * SC).reshape(4, 128, H).transpose(1, 0, 2)
    ).astype(ml_dtypes.float8_e4m3)
    w2p = np.zeros((2, 128, 32), np.float64)
    w2p[:, :, 0:K] = (np.asarray(W2, np.float64) * SC).reshape(2, 128, K)
    w2q = np.ascontiguousarray(w2p.transpose(1, 0, 2)).astype(
        ml_dtypes.float8_e4m3)
    return w1q, w2q


def make_all(x, tags, lengths, W1, b1, W2, b2, trans, start, end):
    x = np.ascontiguousarray(x, np.float32)
    tags = np.ascontiguousarray(tags, np.int32)
    lengths = np.ascontiguousarray(lengths, np.int32)
    assign, na_prof = plan(lengths)
    nc = build(trans, start, end, b1, b2, na_prof)
    split_waits(nc)
    w1q, w2q = quant_weights(W1, W2)
    in_maps = pack_inputs(x, tags, lengths, na_prof, assign)
    for m in in_maps:
        m["w1q"] = w1q
        m["w2q"] = w2q
    return nc, in_maps, assign


def kernel(x, tags, lengths, W1, b1, W2, b2, trans, start, end, trace=False):
    nc, in_maps, assign = make_all(x, tags, lengths, W1, b1, W2, b2,
                                   trans, start, end)
    res = bass_utils.run_bass_kernel_spmd(
        nc, in_maps, core_ids=list(range(NCORES)), trace=trace)
    B = x.shape[0]
    llh = np.zeros(B, np.float64)
    for c in range(NCORES):
        o = res.results[c]["out"].astype(np.float64)  # [2, 128]
        llh[assign[:, c]] = o[:, 0::NE8].reshape(BS)
    loss = np.float32(-(llh.sum()) / float(B))
    if trace:
        return loss, res
    return loss
